# revision 21
# baseline (speedup 1.0000x reference)
"""ChannelWiseProjection Trainium2 kernel.

out[b,c,h,w] = sum_d x[b,h,w,d] * W[c,d] + bias[c]

Strategy: data-parallel over M = b*h*w (65536 rows), 8192 rows per core.
The whole kernel is HBM-bandwidth-bound (x slab is 16 MiB/core in fp32),
so inputs ride in fp16: host pre-transposes each core's x slab to
[D=512, M=8192] (K-major) and downcasts to fp16 — halves the dominant
load — and the output slab is stored fp16 and upcast on host — halves
the store.  Matmuls accumulate in fp32 PSUM, so only the fp16 input
rounding (~1e-3 rel) shows up in the result.  Per core:
out_slab[C=128, M=8192] = W^T-blocked stationary matmuls (fp16, 4
K-blocks accumulated in PSUM) + bias fused into the PSUM->SBUF copy.
Output slabs are channel-major so they DMA straight out and reassemble
into [b, c, h, w] on host.
"""

import numpy as np

from concourse import bacc, mybir, tile
from concourse.bass_utils import run_bass_kernel_spmd

N_CORES = 8
B, H, Wdim, D = 4, 128, 128, 512
C = 128
M_TOT = B * H * Wdim          # 65536
M_CORE = M_TOT // N_CORES     # 8192
KB = D // 128                 # 4 contraction blocks
M_SUB = 512                   # matmul moving size (one PSUM bank, fp32)
# Chunk schedule along M.  Big chunks first: each dma_start costs ~0.65us
# of DIRECT2D issue time on its sequencer, so small leading chunks starve
# the DMA queues during the issue ramp.  Small final chunks minimize the
# serial matmul+add+store tail after the last load byte lands.
CHUNKS = [1024] * 7 + [512, 256, 128, 64, 64]
assert sum(CHUNKS) == M_CORE

_NC = None


def _build():
    global _NC
    if _NC is not None:
        return _NC
    # Bacc (not raw Bass): its finalize() runs the pass pipeline that
    # splits multi-waits into EventSemaphores (TRN2 allows only one sync
    # wait per instruction) — Tile output does not compile without it.
    nc = bacc.Bacc(None)
    # Chunk-blocked layout: for each chunk ci of size s, the block is
    # [128 partitions, KB*s] fp16, fully contiguous per partition row.
    # One 2D DMA with 128 descriptors of KB*s*2 bytes each (8 KiB for a
    # 1024-col chunk) — vs 512 descriptors of 2 KiB under the K-major
    # [KB,128,M] layout, which made each dma_start a ~800ns DIRECT2D on
    # the issuing sequencer and hurt per-queue efficiency.
    xt = nc.declare_dram_parameter(
        "xt", [128, KB * M_CORE], mybir.dt.float16, isOutput=False
    )
    wt = nc.declare_dram_parameter(
        "wt", [128, KB, C], mybir.dt.float16, isOutput=False
    )
    bias = nc.declare_dram_parameter("bias", [C, 1], mybir.dt.float32, isOutput=False)
    out = nc.declare_dram_parameter("out", [C, M_CORE], mybir.dt.float16, isOutput=True)

    with tile.TileContext(nc) as tc:
        with (
            tc.tile_pool(name="const", bufs=1) as cpool,
            tc.tile_pool(name="x", bufs=12) as xpool,
            tc.tile_pool(name="o", bufs=10) as opool,
            tc.tile_pool(name="ps", bufs=8, space="PSUM") as pspool,
        ):
            # w/bias ride the ACT HWDGE ring, which is idle until the first
            # store — they land earlier than via SWDGE, and the first
            # matmul is gated on w's arrival.
            w_sb = cpool.tile([128, KB, C], mybir.dt.float16)
            nc.scalar.dma_start(w_sb[:], wt[:])
            b_sb = cpool.tile([C, 1], mybir.dt.float32)
            nc.scalar.dma_start(b_sb[:], bias[:])

            xt_r = xt[:]
            off = 0
            for ci, size in enumerate(CHUNKS):
                x_sb = xpool.tile([128, KB, size], mybir.dt.float16)
                nc.sync.dma_start(
                    x_sb[:].rearrange("p kb m -> p (kb m)"),
                    xt_r[:, KB * off : KB * (off + size)],
                )
                o_sb = opool.tile([C, size], mybir.dt.float16)
                for ms0 in range(0, size, M_SUB):
                    sub = min(M_SUB, size - ms0)
                    ps = pspool.tile([C, sub], mybir.dt.float32)
                    for kb in range(KB):
                        nc.tensor.matmul(
                            ps[:],
                            w_sb[:, kb, :],
                            x_sb[:, kb, ms0 : ms0 + sub],
                            start=(kb == 0),
                            stop=(kb == KB - 1),
                        )
                    # Bias-add + fp32->fp16 convert on the ACT engine:
                    # out = Copy(ps*1 + bias).  Same engine as the store
                    # that follows, so the store needs no cross-engine
                    # semaphore and the DVE drops out of the graph.
                    nc.scalar.activation(
                        o_sb[:, ms0 : ms0 + sub],
                        ps[:],
                        mybir.ActivationFunctionType.Identity,
                        bias=b_sb[:],
                    )
                # Stores ride the ACT HWDGE ring so they never queue behind
                # the loads on the SP ring.
                nc.scalar.dma_start(out[:, off : off + size], o_sb[:])
                off += size
    nc.finalize()  # Bacc.finalize runs the wait-splitting compile pipeline
    _NC = nc
    return nc


LAST_RESULT = None


def kernel(x, W, b):
    global LAST_RESULT
    nc = _build()

    x = np.asarray(x, dtype=np.float32)
    W = np.asarray(W, dtype=np.float32)
    b = np.asarray(b, dtype=np.float32)

    # Per-core chunk-blocked slabs: for chunk (off, s), columns
    # KB*off : KB*(off+s) hold block[p, kb*s + j] = x[core, off+j, kb*128+p]
    # so each chunk DMA is one contiguous KB*s*2-byte run per partition.
    xr = x.reshape(N_CORES, M_CORE, D)
    xt = np.empty((N_CORES, 128, KB * M_CORE), dtype=np.float16)
    off = 0
    for s in CHUNKS:
        blk = xr[:, off : off + s, :].reshape(N_CORES, s, KB, 128)
        xt[:, :, KB * off : KB * (off + s)] = blk.transpose(0, 3, 2, 1).reshape(
            N_CORES, 128, KB * s
        )
        off += s
    # Stationary weights, blocked: wt[kp, kb, c] = W[c, kb*128 + kp]
    wt = np.ascontiguousarray(W.T.reshape(KB, 128, C).transpose(1, 0, 2)).astype(
        np.float16
    )
    b2 = np.ascontiguousarray(b.reshape(C, 1))

    import os

    in_maps = [{"xt": xt[i], "wt": wt, "bias": b2} for i in range(N_CORES)]
    res = None
    for attempt in range(4):
        try:
            if attempt == 0:
                res = run_bass_kernel_spmd(nc, in_maps, list(range(N_CORES)))
            else:
                # Retry without NTFF tracing: the profile hook's client
                # handle is stale after a backend reset and would raise
                # before the exec even runs.
                os.environ["BASS_NEVER_TRACE"] = "1"
                try:
                    res = run_bass_kernel_spmd(nc, in_maps, list(range(N_CORES)))
                finally:
                    os.environ.pop("BASS_NEVER_TRACE", None)
            break
        except Exception:
            # Transient NRT_EXEC_UNIT_UNRECOVERABLE wedges (stale device
            # state left by a previous process) clear after a backend reset.
            if attempt == 3:
                raise
            try:
                import jax

                jax.clear_caches()
                jax.extend.backend.clear_backends()
                jax.devices()
            except Exception:
                pass
    LAST_RESULT = res

    out = np.empty((B, C, H, Wdim), dtype=np.float32)
    for i in range(N_CORES):
        slab = res.results[i]["out"]  # [C, M_CORE] fp16, m = (h, w)
        bi, half = divmod(i, 2)
        out[bi, :, half * 64 : (half + 1) * 64, :] = slab.astype(np.float32).reshape(
            C, 64, Wdim
        )
    return out
